# revision 13
# baseline (speedup 1.0000x reference)
"""GAT (2-layer graph attention network) forward pass on 8 Trainium2 NeuronCores.

Strategy:
  - Nodes row-partitioned across 8 cores (6250 each); edges partitioned by the
    segment-sum index (src), sorted by (src-block, dst) on host.
  - Per layer, every core builds the full node-feature table
    [h+bias | 1.0 | f2 | f1] (replicated GEMM for layer 1 from the replicated
    input; for layer 2 after an AllGather of x1 shards), stored bf16 in HBM.
  - Edge phase per core: batched row gather (dma_gather) of table rows by dst,
    per-edge attention coef on ACT/DVE, one-hot selector matmul on TensorE
    accumulating [sum_w*h | sum_w] per 128-src-node block in PSUM.
  - Classifier on the local x2 shard; host concatenates shard outputs.
"""

import numpy as np

# ---------------------------------------------------------------------------
# Tile end-drain walrus workaround (this container's walrus rejects >2 sync
# waits per instruction; Tile packs all end-of-kernel waits on one drain).
# ---------------------------------------------------------------------------


def _apply_tile_patch():
    import bass_rust as _br
    from concourse import tile as _tile

    def _patched(self, tick_clock, wait_clock):
        nc = self.nc
        gc = tick_clock.global_clock
        ticks = eval(str(gc).replace("VectorClock(", "").rstrip(")"))
        for p, t in enumerate(ticks):
            if t <= 0:
                continue
            vec = [0] * len(ticks)
            vec[p] = t
            nop = nc.sync.nop(nofuse=True)
            wait_clock.add_sem_waits(
                nop.ins, _br.ScopedClock({None: _br.VectorClock(vec)})
            )
        nc.sync.drain()
        nc.all_engine_barrier()
        assert self.sems is not None
        popped = nc._tile_sem_poison_stack.pop()
        assert popped is self._sem_poison
        nc.clear_and_free_semaphores(list(self.sems.allocated().values()))
        nc.all_engine_barrier()

    _tile.TileContext._drain_and_barrier = _patched


_apply_tile_patch()

import ml_dtypes

import concourse.bacc as bacc
import concourse.bass as bass
import concourse.mybir as mybir
import concourse.tile as tile
from concourse.bass_utils import run_bass_kernel_spmd

BF16 = ml_dtypes.bfloat16
DT = mybir.dt

# problem constants (hardcoded per spec)
N, E = 50000, 1600000
IN, FR, HID, C = 768, 256, 256, 4
ALPHA = 0.2
CORES = 8
NLOC = N // CORES  # 6250
NBLK = (NLOC + 127) // 128  # 49 (last block 106 rows)
ROW = 384  # table row elems (bf16): [h'(256) | 1.0 | f2 | f1 | pad]
HALF = 25088  # table split boundary (128-aligned, both halves < 2^15 rows)
CHUNK = 512  # node rows per GEMM chunk
PHASES = 99  # debug: stop building after this many phases (1=table1, 2=edge1, 3=AG, 4=table2, 5=edge2, 6=cls)


# ---------------------------------------------------------------------------
# Host preprocessing
# ---------------------------------------------------------------------------


def _prep_edges(edges):
    """Partition + sort edges per core; build shared tile schedule and the
    per-core gather metadata arrays."""
    src = np.asarray(edges[0]).astype(np.int64)
    dst = np.asarray(edges[1]).astype(np.int64)
    core = src // NLOC

    per_core = []
    counts = np.zeros((CORES, NBLK, 2), np.int64)
    for c in range(CORES):
        sel = core == c
        s = src[sel] - c * NLOC
        d = dst[sel]
        blk = s >> 7
        half = (d >= HALF).astype(np.int64)
        order = np.lexsort((d, half, blk))
        s, d, blk, half = s[order], d[order], blk[order], half[order]
        grp = blk * 2 + half
        cnt = np.bincount(grp, minlength=NBLK * 2)
        counts[c] = cnt.reshape(NBLK, 2)
        per_core.append((s, d, grp))

    # shared schedule: tiles per (block, half) = max over cores
    maxcnt = counts.max(axis=0)  # [NBLK, 2]
    tiles = (maxcnt + 127) // 128  # [NBLK, 2]
    grp_tiles = tiles.reshape(-1)  # [NBLK*2]
    grp_off = np.zeros(NBLK * 2 + 1, np.int64)
    np.cumsum(grp_tiles, out=grp_off[1:])
    TT = int(grp_off[-1])  # total edge tiles per core

    sched = []
    for b in range(NBLK):
        sched.append(
            dict(
                t_lo=int(tiles[b, 0]),
                t_hi=int(tiles[b, 1]),
                off=int(grp_off[2 * b]),
            )
        )

    metas = []
    for c in range(CORES):
        s, d, grp = per_core[c]
        ne = TT * 128
        dst16 = np.zeros(ne, np.int16)
        slot = np.full(ne, -1.0, np.float32)
        srcl = np.zeros(ne, np.int16)
        # position of each edge within its group
        grp_starts = np.zeros(NBLK * 2, np.int64)
        cnt = np.bincount(grp, minlength=NBLK * 2)
        np.cumsum(cnt[:-1], out=grp_starts[1:])
        pos = np.arange(len(s)) - grp_starts[grp]
        didx = grp_off[grp] * 128 + pos
        half = grp & 1
        dst16[didx] = (d - half * HALF).astype(np.int16)
        slot[didx] = (s & 127).astype(np.float32)
        srcl[didx] = s.astype(np.int16)

        def wrap16(a):
            w = np.zeros((128, 8 * TT), np.int16)
            w16 = a.reshape(-1, 16).T  # [16, 8*TT]
            for g in range(8):
                w[g * 16 : (g + 1) * 16, :] = w16
            return w

        slotmat = np.ascontiguousarray(slot.reshape(TT, 128).T).astype(BF16)
        metas.append(
            dict(wrap=wrap16(dst16), srcwrap=wrap16(srcl), slotmat=slotmat)
        )

    return sched, TT, metas


def _prep_weights(inp):
    """Host-side weight transforms (all tiny)."""
    f = lambda a: np.asarray(a, np.float32)
    out = {}
    for L, (wl, ws, a1, b1, a2, b2, bi) in {
        1: ("W_lin1", "W_seq1", "a1_1", "b1_1", "a2_1", "b2_1", "bias1"),
        2: ("W_lin2", "W_seq2", "a1_2", "b1_2", "a2_2", "b2_2", "bias2"),
    }.items():
        WL, WS = f(inp[wl]), f(inp[ws])
        a1v, a2v = f(inp[a1]), f(inp[a2])
        b1v, b2v = np.float32(inp[b1]), np.float32(inp[b2])
        bias = f(inp[bi])
        out[f"w{L}t"] = np.ascontiguousarray(WL.T).astype(BF16)  # [K, FR]
        out[f"ws{L}t"] = np.ascontiguousarray(WS.T).astype(BF16)  # [FR, HID]
        # f1 = t @ (WS.T @ a1) + b1 ; table cols ordered [.. f2 | f1]
        at1 = WS.T @ a1v
        at2 = WS.T @ a2v
        out[f"apack{L}"] = np.ascontiguousarray(
            np.stack([at2, at1], axis=1)
        ).astype(BF16)  # [FR, 2] = [a~2 | a~1]
        out[f"fb{L}"] = np.array([[b2v, b1v]], np.float32).astype(BF16)  # [1,2]
        out[f"brow{L}"] = bias[None, :].astype(BF16)  # [1, HID]
    out["wct"] = np.ascontiguousarray(f(inp["W_cls"]).T).astype(BF16)  # [HID, C]
    out["bcrow"] = f(inp["b_cls"])[None, :].astype(BF16)  # [1, C]
    out["iota"] = np.tile(
        np.arange(128, dtype=np.float32)[None, :], (128, 1)
    ).astype(BF16)
    return out


# ---------------------------------------------------------------------------
# Device program
# ---------------------------------------------------------------------------


def _load_consts(tc, nc, pool, x_in, TT):
    """Load weights + edge metadata into resident SBUF tiles."""
    t = {}

    def dma(name, shape, dtype, ap):
        tl = pool.tile(shape, dtype, tag=name)
        nc.sync.dma_start(out=tl[:], in_=ap)
        t[name] = tl

    dma("iota", [128, 128], DT.bfloat16, x_in["iota"][:, :])
    dma("slotmat", [128, TT], DT.bfloat16, x_in["slotmat"][:, :])

    dma(
        "w1t", [128, IN // 128, FR], DT.bfloat16,
        x_in["w1t"].rearrange("(k p) m -> p k m", p=128),
    )
    dma(
        "w2t", [128, HID // 128, FR], DT.bfloat16,
        x_in["w2t"].rearrange("(k p) m -> p k m", p=128),
    )
    for L in (1, 2):
        dma(
            f"ws{L}t", [128, FR // 128, HID], DT.bfloat16,
            x_in[f"ws{L}t"].rearrange("(k p) m -> p k m", p=128),
        )
        dma(
            f"apack{L}", [128, FR // 128, 2], DT.bfloat16,
            x_in[f"apack{L}"].rearrange("(k p) m -> p k m", p=128),
        )
        dma(f"brow{L}", [1, HID], DT.bfloat16, x_in[f"brow{L}"][:, :])
        dma(f"fb{L}", [1, 2], DT.bfloat16, x_in[f"fb{L}"][:, :])
    dma(
        "wct", [128, HID // 128, C], DT.bfloat16,
        x_in["wct"].rearrange("(k p) m -> p k m", p=128),
    )
    dma("bcrow", [1, C], DT.bfloat16, x_in["bcrow"][:, :])

    ones = pool.tile([1, 128], DT.bfloat16)
    nc.vector.memset(ones[:], 1.0)
    t["ones"] = ones
    return t


def _table_phase(tc, nc, cst, L, rhs_loader, k_chunks, nrows, tableL, tableH, f1full):
    """Build the per-node table [h+bias | 1 | f2 | f1] for `nrows` nodes
    (chunked GEMMs), writing table rows + f1 column to DRAM."""
    wT = cst[f"w{L}t"]
    wsT = cst[f"ws{L}t"]
    apack = cst[f"apack{L}"]
    brow = cst[f"brow{L}"]
    fb = cst[f"fb{L}"]
    ones = cst["ones"]

    nchunks = (nrows + CHUNK - 1) // CHUNK
    with (
        tc.tile_pool(name=f"tp{L}_rhs", bufs=2) as rhs_pool,
        tc.tile_pool(name=f"tp{L}_t", bufs=2) as t_pool,
        tc.tile_pool(name=f"tp{L}_asm", bufs=3) as asm_pool,
        tc.tile_pool(name=f"tp{L}_ps1", bufs=2, space="PSUM") as ps1,
        tc.tile_pool(name=f"tp{L}_ps2", bufs=2, space="PSUM") as ps2,
        tc.tile_pool(name=f"tp{L}_psf", bufs=2, space="PSUM") as psf,
    ):
        for ci in range(nchunks):
            r0 = ci * CHUNK
            crows = min(CHUNK, nrows - r0)
            # G1: tT[fr, node] = W_lin @ x.T   (K = in-features)
            rhs_tiles = [rhs_loader(rhs_pool, ci, k, crows) for k in range(k_chunks)]
            tT = t_pool.tile([128, FR // 128, CHUNK], DT.bfloat16, tag="tT")
            for m in range(FR // 128):
                pt = ps1.tile([128, CHUNK], DT.float32, space="PSUM", tag="g1")
                for k in range(k_chunks):
                    nc.tensor.matmul(
                        out=pt[:, :crows],
                        lhsT=wT[:, k, m * 128 : (m + 1) * 128],
                        rhs=rhs_tiles[k][:, :crows],
                        start=(k == 0),
                        stop=(k == k_chunks - 1),
                    )
                nc.scalar.copy(out=tT[:, m, :crows], in_=pt[:, :crows])
            # G2 per 128-node block
            nblocks = (crows + 127) // 128
            for bi in range(nblocks):
                boff = bi * 128
                brows = min(128, crows - boff)
                hp = ps2.tile([128, HID], DT.float32, space="PSUM", tag="g2")
                nc.tensor.matmul(
                    out=hp[:brows, :],
                    lhsT=tT[:, 0, boff : boff + brows],
                    rhs=wsT[:, 0, :],
                    start=True,
                    stop=False,
                )
                nc.tensor.matmul(
                    out=hp[:brows, :],
                    lhsT=tT[:, 1, boff : boff + brows],
                    rhs=wsT[:, 1, :],
                    start=False,
                    stop=False,
                )
                nc.tensor.matmul(
                    out=hp[:brows, :],
                    lhsT=ones[0:1, 0:brows],
                    rhs=brow[0:1, :],
                    start=False,
                    stop=True,
                )
                fp = psf.tile([128, 2], DT.float32, space="PSUM", tag="gf")
                nc.tensor.matmul(
                    out=fp[:brows, :],
                    lhsT=tT[:, 0, boff : boff + brows],
                    rhs=apack[:, 0, :],
                    start=True,
                    stop=False,
                )
                nc.tensor.matmul(
                    out=fp[:brows, :],
                    lhsT=tT[:, 1, boff : boff + brows],
                    rhs=apack[:, 1, :],
                    start=False,
                    stop=False,
                )
                nc.tensor.matmul(
                    out=fp[:brows, :],
                    lhsT=ones[0:1, 0:brows],
                    rhs=fb[0:1, :],
                    start=False,
                    stop=True,
                )
                hb = asm_pool.tile([128, ROW], DT.bfloat16, tag="hb")
                nc.scalar.copy(out=hb[:brows, 0:HID], in_=hp[:brows, :])
                nc.vector.memset(hb[:brows, HID:ROW], 1.0)
                nc.vector.tensor_copy(
                    out=hb[:brows, HID + 1 : HID + 3], in_=fp[:brows, :]
                )
                f1c = asm_pool.tile([128, 1], DT.float32, tag="f1c")
                nc.vector.tensor_copy(out=f1c[:brows, :], in_=fp[:brows, 1:2])
                g0 = r0 + boff
                tdst = tableL if g0 < HALF else tableH
                goff = g0 if g0 < HALF else g0 - HALF
                nc.sync.dma_start(
                    out=tdst[goff : goff + brows, :], in_=hb[:brows, :]
                )
                nc.sync.dma_start(
                    out=f1full[g0 : g0 + brows, :], in_=f1c[:brows, :]
                )


def _edge_phase(tc, nc, cst, L, sched, x_in, tableL, tableH, f1tab, out_writer):
    """Per src-block: gather table rows by dst, build weighted one-hot S,
    accumulate [sum_w*h' | sum_w] in PSUM via matmul, normalize."""
    iota = cst["iota"]
    slotmat = cst["slotmat"]
    wrap_d = x_in["wrap"]
    srcwrap_d = x_in["srcwrap"]
    TMAX = max(s["t_lo"] + s["t_hi"] for s in sched)

    with (
        tc.tile_pool(name=f"ep{L}_g", bufs=3) as gpool,
        tc.tile_pool(name=f"ep{L}_i", bufs=3) as ipool,
        tc.tile_pool(name=f"ep{L}_c", bufs=3) as cpool,
        tc.tile_pool(name=f"ep{L}_s", bufs=2) as spool,
        tc.tile_pool(name=f"ep{L}_o", bufs=3) as opool,
        tc.tile_pool(name=f"ep{L}_ps", bufs=2, space="PSUM") as pspool,
    ):
        for b, sc in enumerate(sched):
            t_lo, t_hi, off = sc["t_lo"], sc["t_hi"], sc["off"]
            t_b = t_lo + t_hi
            if t_b == 0:
                continue
            brows = min(128, NLOC - b * 128)
            groups = []
            if t_lo:
                ilo = ipool.tile([128, 8 * t_lo], DT.int16, tag="ilo")
                nc.sync.dma_start(
                    out=ilo[:], in_=wrap_d[:, 8 * off : 8 * (off + t_lo)]
                )
                g_lo = gpool.tile([128, t_lo, ROW], DT.bfloat16, tag="glo")
                nc.gpsimd.dma_gather(
                    out_ap=g_lo[:],
                    in_ap=tableL.ap(),
                    idxs_ap=ilo[:],
                    num_idxs=t_lo * 128,
                    num_idxs_reg=t_lo * 128,
                    elem_size=ROW,
                    single_packet=False,
                    queue_num=(3 * b) % 4,
                )
                groups.append((g_lo, t_lo))
            if t_hi:
                ihi = ipool.tile([128, 8 * t_hi], DT.int16, tag="ihi")
                nc.sync.dma_start(
                    out=ihi[:], in_=wrap_d[:, 8 * (off + t_lo) : 8 * (off + t_b)]
                )
                g_hi = gpool.tile([128, t_hi, ROW], DT.bfloat16, tag="ghi")
                nc.gpsimd.dma_gather(
                    out_ap=g_hi[:],
                    in_ap=tableH.ap(),
                    idxs_ap=ihi[:],
                    num_idxs=t_hi * 128,
                    num_idxs_reg=t_hi * 128,
                    elem_size=ROW,
                    single_packet=False,
                    queue_num=(3 * b + 1) % 4,
                )
                groups.append((g_hi, t_hi))

            if1 = ipool.tile([128, 8 * t_b], DT.int16, tag="if1")
            nc.sync.dma_start(out=if1[:], in_=srcwrap_d[:, 8 * off : 8 * (off + t_b)])
            gf1 = gpool.tile([128, t_b, 128], DT.bfloat16, tag="gf1")
            nc.gpsimd.dma_gather(
                out_ap=gf1[:],
                in_ap=f1tab.ap(),
                idxs_ap=if1[:],
                num_idxs=t_b * 128,
                num_idxs_reg=t_b * 128,
                elem_size=128,
                single_packet=False,
                queue_num=(3 * b + 2) % 4,
            )
            f1sb = cpool.tile([128, TMAX], DT.float32, tag="f1sb")
            nc.vector.tensor_copy(out=f1sb[:, 0:t_b], in_=gf1[:, :, 0])
            # coef = exp(lrelu(f1 + f2))
            logits = cpool.tile([128, TMAX], DT.float32, tag="logits")
            tcol = 0
            for g, tn in groups:
                nc.vector.tensor_copy(
                    out=logits[:, tcol : tcol + tn], in_=g[:, :, HID + 1]
                )
                tcol += tn
            nc.vector.tensor_add(
                out=logits[:, 0:t_b], in0=logits[:, 0:t_b], in1=f1sb[:, 0:t_b]
            )
            coef = cpool.tile([128, TMAX], DT.float32, tag="coef")
            nc.vector.tensor_scalar(
                out=coef[:, 0:t_b], in0=logits[:, 0:t_b], scalar1=ALPHA,
                scalar2=None, op0=mybir.AluOpType.mult,
            )
            nc.vector.tensor_tensor(
                out=coef[:, 0:t_b], in0=coef[:, 0:t_b], in1=logits[:, 0:t_b],
                op=mybir.AluOpType.max,
            )
            nc.scalar.activation(
                out=coef[:, 0:t_b], in_=coef[:, 0:t_b],
                func=mybir.ActivationFunctionType.Exp,
            )

            # batched one-hot S for the whole block: S[:, t*128+j] =
            # (slot[p,t]==j) * coef[p,t]
            coefb = cpool.tile([128, TMAX], DT.bfloat16, tag="coefb")
            nc.vector.tensor_copy(out=coefb[:, 0:t_b], in_=coef[:, 0:t_b])
            S = spool.tile([128, t_b * 128], DT.bfloat16, tag="S")
            s_ap = S[:].rearrange("p (t j) -> p t j", t=t_b)
            sm = slotmat[:]
            slot_ap = bass.AP(
                sm.tensor, sm.offset + off, [list(sm.ap[0]), [1, t_b], [0, 128]]
            )
            io = iota[:]
            iota_ap = bass.AP(
                io.tensor, io.offset, [list(io.ap[0]), [0, t_b], [1, 128]]
            )
            nc.vector.tensor_tensor(
                out=s_ap, in0=slot_ap, in1=iota_ap, op=mybir.AluOpType.is_equal
            )
            cb = coefb[:]
            coef_ap = bass.AP(
                cb.tensor, cb.offset, [list(cb.ap[0]), [1, t_b], [0, 128]]
            )
            nc.vector.tensor_tensor(
                out=s_ap, in0=s_ap, in1=coef_ap, op=mybir.AluOpType.mult
            )

            acc = pspool.tile([128, HID + 1], DT.float32, space="PSUM", tag="acc")
            t_glob = 0
            for g, tn in groups:
                for t in range(tn):
                    nc.tensor.matmul(
                        out=acc[:],
                        lhsT=S[:, t_glob * 128 : (t_glob + 1) * 128],
                        rhs=g[:, t, 0 : HID + 1],
                        start=(t_glob == 0),
                        stop=(t_glob == t_b - 1),
                    )
                    t_glob += 1

            recip = opool.tile([128, 1], DT.float32, tag="recip")
            nc.vector.tensor_scalar(
                out=recip[:], in0=acc[:, HID : HID + 1], scalar1=1e-30,
                scalar2=None, op0=mybir.AluOpType.add,
            )
            nc.vector.reciprocal(out=recip[:], in_=recip[:])
            out_writer(opool, b, brows, acc, recip)


def _build_program(sched, TT):
    nc = bacc.Bacc(
        "TRN2", target_bir_lowering=False, debug=False, num_devices=CORES,
        enable_partition_id=True, num_swdge_queues=4,
    )
    x_in = {}

    def param(name, shape, dtype):
        x_in[name] = nc.declare_dram_parameter(name, list(shape), dtype, isOutput=False)

    param("xT", [IN, N], DT.bfloat16)
    param("wrap", [128, 8 * TT], DT.int16)
    param("srcwrap", [128, 8 * TT], DT.int16)
    param("slotmat", [128, TT], DT.bfloat16)
    param("w1t", [IN, FR], DT.bfloat16)
    param("w2t", [HID, FR], DT.bfloat16)
    for L in (1, 2):
        param(f"ws{L}t", [FR, HID], DT.bfloat16)
        param(f"apack{L}", [FR, 2], DT.bfloat16)
        param(f"brow{L}", [1, HID], DT.bfloat16)
        param(f"fb{L}", [1, 2], DT.bfloat16)
    param("wct", [HID, C], DT.bfloat16)
    param("bcrow", [1, C], DT.bfloat16)
    param("iota", [128, 128], DT.bfloat16)

    x2_out = nc.declare_dram_parameter("x2", [NLOC, HID], DT.float32, isOutput=True)
    cls_out = nc.declare_dram_parameter("cls", [NLOC, C], DT.float32, isOutput=True)

    # internal DRAM (own tensors: offset-0 for indirect/gather bases)
    tableL1 = nc.dram_tensor("tableL1", [HALF, ROW], DT.bfloat16)
    tableH1 = nc.dram_tensor("tableH1", [N - HALF, ROW], DT.bfloat16)
    tableL2 = nc.dram_tensor("tableL2", [HALF, ROW], DT.bfloat16)
    tableH2 = nc.dram_tensor("tableH2", [N - HALF, ROW], DT.bfloat16)
    f1full1 = nc.dram_tensor("f1full1", [N, 1], DT.float32)
    f1full2 = nc.dram_tensor("f1full2", [N, 1], DT.float32)
    F1ROWS = NBLK * 128
    f1tab1 = nc.dram_tensor("f1tab1", [F1ROWS, 128], DT.bfloat16)
    f1tab2 = nc.dram_tensor("f1tab2", [F1ROWS, 128], DT.bfloat16)
    x1_local = nc.dram_tensor("x1_local", [NLOC, HID], DT.bfloat16)
    x1_full = nc.dram_tensor("x1_full", [N, HID], DT.bfloat16)
    x2_bf = nc.dram_tensor("x2_bf", [NBLK * 128, HID], DT.bfloat16)

    with tile.TileContext(nc) as tc:
        pid = nc.partition_id()

        def localize_f1(L, f1full, f1tab):
            with tc.tile_pool(name=f"lc{L}", bufs=3) as lp:
                for b in range(NBLK):
                    brows = min(128, NLOC - b * 128)
                    f1c = lp.tile([128, 1], DT.float32, tag="f1c")
                    offv = nc.s_assert_within(
                        pid * NLOC + 128 * b,
                        128 * b,
                        (CORES - 1) * NLOC + 128 * b,
                        skip_runtime_assert=True,
                    )
                    nc.sync.dma_start(
                        out=f1c[:brows, :], in_=f1full[bass.ds(offv, brows), :]
                    )
                    rep = lp.tile([128, 128], DT.bfloat16, tag="rep")
                    if brows < 128:
                        nc.vector.memset(rep[:], 0.0)
                    nc.vector.tensor_copy(
                        out=rep[:brows, :],
                        in_=f1c[:brows, 0:1].to_broadcast([brows, 128]),
                    )
                    nc.sync.dma_start(
                        out=f1tab[b * 128 : (b + 1) * 128, :], in_=rep[:, :]
                    )

        with tc.tile_pool(name="const", bufs=1) as cstpool:
            cst = _load_consts(tc, nc, cstpool, x_in, TT)

            # ---- layer 1 table (replicated: all N rows) ----
            def rhs_l1(pool, ci, k, crows):
                t = pool.tile([128, CHUNK], DT.bfloat16, tag=f"rhs{k}")
                nc.sync.dma_start(
                    out=t[:, :crows],
                    in_=x_in["xT"][
                        k * 128 : (k + 1) * 128, ci * CHUNK : ci * CHUNK + crows
                    ],
                )
                return t

            _table_phase(
                tc, nc, cst, 1, rhs_l1, IN // 128, N, tableL1, tableH1, f1full1
            )
            localize_f1(1, f1full1, f1tab1)
            if PHASES < 2:
                return nc

            # ---- layer 1 edge phase -> x1 shard ----
            def write_x1(opool, b, brows, acc, recip):
                xb = opool.tile([128, HID], DT.bfloat16, tag="xb")
                nc.vector.tensor_scalar(
                    out=xb[:brows, :], in0=acc[:brows, 0:HID],
                    scalar1=recip[:brows, :], scalar2=None,
                    op0=mybir.AluOpType.mult,
                )
                nc.sync.dma_start(
                    out=x1_local[b * 128 : b * 128 + brows, :], in_=xb[:brows, :]
                )

            _edge_phase(tc, nc, cst, 1, sched, x_in, tableL1, tableH1, f1tab1, write_x1)
            if PHASES < 3:
                return nc

            # ---- all-gather x1 shards ----
            nc.gpsimd.collective_compute(
                "AllGather",
                mybir.AluOpType.bypass,
                replica_groups=[list(range(CORES))],
                ins=[x1_local.ap().opt()],
                outs=[x1_full.ap().opt()],
            )
            if PHASES < 4:
                return nc

            # ---- layer 2 table (replicated, from gathered x1) ----
            def rhs_l2(pool, ci, k, crows):
                t = pool.tile([128, CHUNK], DT.bfloat16, tag=f"rhs{k}")
                nc.sync.dma_start(
                    out=t[:, :crows],
                    in_=x1_full[
                        ci * CHUNK : ci * CHUNK + crows, k * 128 : (k + 1) * 128
                    ],
                    transpose=True,
                )
                return t

            _table_phase(
                tc, nc, cst, 2, rhs_l2, HID // 128, N, tableL2, tableH2, f1full2
            )
            localize_f1(2, f1full2, f1tab2)
            if PHASES < 5:
                return nc

            # ---- layer 2 edge phase -> x2 outputs ----
            def write_x2(opool, b, brows, acc, recip):
                xf = opool.tile([128, HID], DT.float32, tag="xf")
                nc.vector.tensor_scalar(
                    out=xf[:brows, :], in0=acc[:brows, 0:HID],
                    scalar1=recip[:brows, :], scalar2=None,
                    op0=mybir.AluOpType.mult,
                )
                nc.sync.dma_start(
                    out=x2_out[b * 128 : b * 128 + brows, :], in_=xf[:brows, :]
                )
                xb = opool.tile([128, HID], DT.bfloat16, tag="xb2")
                nc.vector.tensor_copy(out=xb[:brows, :], in_=xf[:brows, :])
                nc.sync.dma_start(
                    out=x2_bf[b * 128 : b * 128 + brows, :], in_=xb[:brows, :]
                )

            _edge_phase(tc, nc, cst, 2, sched, x_in, tableL2, tableH2, f1tab2, write_x2)
            if PHASES < 6:
                return nc

            # ---- classifier ----
            with (
                tc.tile_pool(name="cl_x", bufs=3) as clx,
                tc.tile_pool(name="cl_o", bufs=3) as clo,
                tc.tile_pool(name="cl_ps", bufs=2, space="PSUM") as clps,
            ):
                if NLOC % 128:
                    zt = clx.tile([128, HID], DT.bfloat16, tag="zt")
                    nc.vector.memset(zt[:], 0.0)
                    nc.sync.dma_start(
                        out=x2_bf[NLOC : NBLK * 128, :],
                        in_=zt[0 : NBLK * 128 - NLOC, :],
                    )
                wct = cst["wct"]
                bcrow = cst["bcrow"]
                ones = cst["ones"]
                for b in range(NBLK):
                    brows = min(128, NLOC - b * 128)
                    brows16 = (brows + 15) // 16 * 16
                    cp = clps.tile([128, C], DT.float32, space="PSUM", tag="cp")
                    for k in range(HID // 128):
                        xt = clx.tile([128, 128], DT.bfloat16, tag="xt")
                        nc.sync.dma_start(
                            out=xt[:, 0:brows16],
                            in_=x2_bf[
                                b * 128 : b * 128 + brows16, k * 128 : (k + 1) * 128
                            ],
                            transpose=True,
                        )
                        nc.tensor.matmul(
                            out=cp[:brows, :],
                            lhsT=xt[:, 0:brows],
                            rhs=wct[:, k, :],
                            start=(k == 0),
                            stop=False,
                        )
                    nc.tensor.matmul(
                        out=cp[:brows, :],
                        lhsT=ones[0:1, 0:brows],
                        rhs=bcrow[0:1, :],
                        start=False,
                        stop=True,
                    )
                    co = clo.tile([128, C], DT.float32, tag="co")
                    nc.vector.tensor_copy(out=co[:brows, :], in_=cp[:brows, :])
                    nc.sync.dma_start(
                        out=cls_out[b * 128 : b * 128 + brows, :], in_=co[:brows, :]
                    )

    return nc


# ---------------------------------------------------------------------------
# Entry point
# ---------------------------------------------------------------------------

_RUN_KWARGS = {}  # test.py can inject trace=True


def kernel(**inputs):
    x = np.asarray(inputs["x"], np.float32)
    edges = np.asarray(inputs["edges"])

    sched, TT, metas = _prep_edges(edges)
    wts = _prep_weights(inputs)
    xT = np.ascontiguousarray(x.T).astype(BF16)

    nc = _build_program(sched, TT)
    nc.compile()

    shared = {k: v for k, v in wts.items()}
    shared["xT"] = xT
    in_maps = []
    for c in range(CORES):
        m = dict(shared)
        m.update(metas[c])
        in_maps.append(m)

    res = run_bass_kernel_spmd(
        nc, in_maps, core_ids=list(range(CORES)), **_RUN_KWARGS
    )
    kernel._last_results = res

    x2 = np.concatenate(
        [np.asarray(res.results[c]["x2"], np.float32) for c in range(CORES)], axis=0
    )
    out = np.concatenate(
        [np.asarray(res.results[c]["cls"], np.float32) for c in range(CORES)], axis=0
    )
    return (out, x2)


# revision 14
# speedup vs baseline: 1.0193x; 1.0193x over previous
"""GAT (2-layer graph attention network) forward pass on 8 Trainium2 NeuronCores.

Strategy:
  - Nodes row-partitioned across 8 cores (6250 each); edges partitioned by the
    segment-sum index (src), sorted by (src-block, dst) on host.
  - Per layer, every core builds the full node-feature table
    [h+bias | 1.0 | f2 | f1] (replicated GEMM for layer 1 from the replicated
    input; for layer 2 after an AllGather of x1 shards), stored bf16 in HBM.
  - Edge phase per core: batched row gather (dma_gather) of table rows by dst,
    per-edge attention coef on ACT/DVE, one-hot selector matmul on TensorE
    accumulating [sum_w*h | sum_w] per 128-src-node block in PSUM.
  - Classifier on the local x2 shard; host concatenates shard outputs.
"""

import numpy as np

# ---------------------------------------------------------------------------
# Tile end-drain walrus workaround (this container's walrus rejects >2 sync
# waits per instruction; Tile packs all end-of-kernel waits on one drain).
# ---------------------------------------------------------------------------


def _apply_tile_patch():
    import bass_rust as _br
    from concourse import tile as _tile

    def _patched(self, tick_clock, wait_clock):
        nc = self.nc
        gc = tick_clock.global_clock
        ticks = eval(str(gc).replace("VectorClock(", "").rstrip(")"))
        for p, t in enumerate(ticks):
            if t <= 0:
                continue
            vec = [0] * len(ticks)
            vec[p] = t
            nop = nc.sync.nop(nofuse=True)
            wait_clock.add_sem_waits(
                nop.ins, _br.ScopedClock({None: _br.VectorClock(vec)})
            )
        nc.sync.drain()
        nc.all_engine_barrier()
        assert self.sems is not None
        popped = nc._tile_sem_poison_stack.pop()
        assert popped is self._sem_poison
        nc.clear_and_free_semaphores(list(self.sems.allocated().values()))
        nc.all_engine_barrier()

    _tile.TileContext._drain_and_barrier = _patched


_apply_tile_patch()

import ml_dtypes

import concourse.bacc as bacc
import concourse.bass as bass
import concourse.mybir as mybir
import concourse.tile as tile
from concourse.bass_utils import run_bass_kernel_spmd

BF16 = ml_dtypes.bfloat16
DT = mybir.dt

# problem constants (hardcoded per spec)
N, E = 50000, 1600000
IN, FR, HID, C = 768, 256, 256, 4
ALPHA = 0.2
CORES = 8
NLOC = N // CORES  # 6250
NBLK = (NLOC + 127) // 128  # 49 (last block 106 rows)
ROW = 384  # table row elems (bf16): [h'(256) | 1.0 | f2 | f1 | pad]
HALF = 25088  # table split boundary (128-aligned, both halves < 2^15 rows)
CHUNK = 512  # node rows per GEMM chunk
PHASES = 99  # debug: stop building after this many phases (1=table1, 2=edge1, 3=AG, 4=table2, 5=edge2, 6=cls)


# ---------------------------------------------------------------------------
# Host preprocessing
# ---------------------------------------------------------------------------


def _prep_edges(edges):
    """Partition + sort edges per core; build shared tile schedule and the
    per-core gather metadata arrays."""
    src = np.asarray(edges[0]).astype(np.int64)
    dst = np.asarray(edges[1]).astype(np.int64)
    core = src // NLOC

    per_core = []
    counts = np.zeros((CORES, NBLK, 2), np.int64)
    for c in range(CORES):
        sel = core == c
        s = src[sel] - c * NLOC
        d = dst[sel]
        blk = s >> 7
        half = (d >= HALF).astype(np.int64)
        order = np.lexsort((d, half, blk))
        s, d, blk, half = s[order], d[order], blk[order], half[order]
        grp = blk * 2 + half
        cnt = np.bincount(grp, minlength=NBLK * 2)
        counts[c] = cnt.reshape(NBLK, 2)
        per_core.append((s, d, grp))

    # shared schedule: tiles per (block, half) = max over cores
    maxcnt = counts.max(axis=0)  # [NBLK, 2]
    tiles = (maxcnt + 127) // 128  # [NBLK, 2]
    grp_tiles = tiles.reshape(-1)  # [NBLK*2]
    grp_off = np.zeros(NBLK * 2 + 1, np.int64)
    np.cumsum(grp_tiles, out=grp_off[1:])
    TT = int(grp_off[-1])  # total edge tiles per core

    sched = []
    for b in range(NBLK):
        sched.append(
            dict(
                t_lo=int(tiles[b, 0]),
                t_hi=int(tiles[b, 1]),
                off=int(grp_off[2 * b]),
            )
        )

    metas = []
    for c in range(CORES):
        s, d, grp = per_core[c]
        ne = TT * 128
        dst16 = np.zeros(ne, np.int16)
        slot = np.full(ne, -1.0, np.float32)
        srcl = np.zeros(ne, np.int16)
        # position of each edge within its group
        grp_starts = np.zeros(NBLK * 2, np.int64)
        cnt = np.bincount(grp, minlength=NBLK * 2)
        np.cumsum(cnt[:-1], out=grp_starts[1:])
        pos = np.arange(len(s)) - grp_starts[grp]
        didx = grp_off[grp] * 128 + pos
        half = grp & 1
        dst16[didx] = (d - half * HALF).astype(np.int16)
        slot[didx] = (s & 127).astype(np.float32)
        srcl[didx] = s.astype(np.int16)

        def wrap16(a):
            w = np.zeros((128, 8 * TT), np.int16)
            w16 = a.reshape(-1, 16).T  # [16, 8*TT]
            for g in range(8):
                w[g * 16 : (g + 1) * 16, :] = w16
            return w

        slotmat = np.ascontiguousarray(slot.reshape(TT, 128).T).astype(BF16)
        metas.append(
            dict(wrap=wrap16(dst16), srcwrap=wrap16(srcl), slotmat=slotmat)
        )

    return sched, TT, metas


def _prep_weights(inp):
    """Host-side weight transforms (all tiny)."""
    f = lambda a: np.asarray(a, np.float32)
    out = {}
    for L, (wl, ws, a1, b1, a2, b2, bi) in {
        1: ("W_lin1", "W_seq1", "a1_1", "b1_1", "a2_1", "b2_1", "bias1"),
        2: ("W_lin2", "W_seq2", "a1_2", "b1_2", "a2_2", "b2_2", "bias2"),
    }.items():
        WL, WS = f(inp[wl]), f(inp[ws])
        a1v, a2v = f(inp[a1]), f(inp[a2])
        b1v, b2v = np.float32(inp[b1]), np.float32(inp[b2])
        bias = f(inp[bi])
        out[f"w{L}t"] = np.ascontiguousarray(WL.T).astype(BF16)  # [K, FR]
        out[f"ws{L}t"] = np.ascontiguousarray(WS.T).astype(BF16)  # [FR, HID]
        # f1 = t @ (WS.T @ a1) + b1 ; table cols ordered [.. f2 | f1]
        at1 = WS.T @ a1v
        at2 = WS.T @ a2v
        out[f"apack{L}"] = np.ascontiguousarray(
            np.stack([at2, at1], axis=1)
        ).astype(BF16)  # [FR, 2] = [a~2 | a~1]
        out[f"fb{L}"] = np.array([[b2v, b1v]], np.float32).astype(BF16)  # [1,2]
        out[f"brow{L}"] = bias[None, :].astype(BF16)  # [1, HID]
    out["wct"] = np.ascontiguousarray(f(inp["W_cls"]).T).astype(BF16)  # [HID, C]
    out["bcrow"] = f(inp["b_cls"])[None, :].astype(BF16)  # [1, C]
    out["iota"] = np.tile(
        np.arange(128, dtype=np.float32)[None, :], (128, 1)
    ).astype(BF16)
    return out


# ---------------------------------------------------------------------------
# Device program
# ---------------------------------------------------------------------------


def _load_consts(tc, nc, pool, x_in, TT):
    """Load weights + edge metadata into resident SBUF tiles."""
    t = {}

    def dma(name, shape, dtype, ap):
        tl = pool.tile(shape, dtype, tag=name)
        nc.sync.dma_start(out=tl[:], in_=ap)
        t[name] = tl

    dma("iota", [128, 128], DT.bfloat16, x_in["iota"][:, :])
    dma("slotmat", [128, TT], DT.bfloat16, x_in["slotmat"][:, :])

    dma(
        "w1t", [128, IN // 128, FR], DT.bfloat16,
        x_in["w1t"].rearrange("(k p) m -> p k m", p=128),
    )
    dma(
        "w2t", [128, HID // 128, FR], DT.bfloat16,
        x_in["w2t"].rearrange("(k p) m -> p k m", p=128),
    )
    for L in (1, 2):
        dma(
            f"ws{L}t", [128, FR // 128, HID], DT.bfloat16,
            x_in[f"ws{L}t"].rearrange("(k p) m -> p k m", p=128),
        )
        dma(
            f"apack{L}", [128, FR // 128, 2], DT.bfloat16,
            x_in[f"apack{L}"].rearrange("(k p) m -> p k m", p=128),
        )
        dma(f"brow{L}", [1, HID], DT.bfloat16, x_in[f"brow{L}"][:, :])
        dma(f"fb{L}", [1, 2], DT.bfloat16, x_in[f"fb{L}"][:, :])
    dma(
        "wct", [128, HID // 128, C], DT.bfloat16,
        x_in["wct"].rearrange("(k p) m -> p k m", p=128),
    )
    dma("bcrow", [1, C], DT.bfloat16, x_in["bcrow"][:, :])

    ones = pool.tile([1, 128], DT.bfloat16)
    nc.vector.memset(ones[:], 1.0)
    t["ones"] = ones
    return t


def _table_phase(tc, nc, cst, L, rhs_loader, k_chunks, nrows, tableL, tableH, f1full):
    """Build the per-node table [h+bias | 1 | f2 | f1] for `nrows` nodes
    (chunked GEMMs), writing table rows + f1 column to DRAM."""
    wT = cst[f"w{L}t"]
    wsT = cst[f"ws{L}t"]
    apack = cst[f"apack{L}"]
    brow = cst[f"brow{L}"]
    fb = cst[f"fb{L}"]
    ones = cst["ones"]

    nchunks = (nrows + CHUNK - 1) // CHUNK
    with (
        tc.tile_pool(name=f"tp{L}_rhs", bufs=2) as rhs_pool,
        tc.tile_pool(name=f"tp{L}_t", bufs=2) as t_pool,
        tc.tile_pool(name=f"tp{L}_asm", bufs=3) as asm_pool,
        tc.tile_pool(name=f"tp{L}_ps1", bufs=2, space="PSUM") as ps1,
        tc.tile_pool(name=f"tp{L}_ps2", bufs=2, space="PSUM") as ps2,
        tc.tile_pool(name=f"tp{L}_psf", bufs=2, space="PSUM") as psf,
    ):
        for ci in range(nchunks):
            r0 = ci * CHUNK
            crows = min(CHUNK, nrows - r0)
            # G1: tT[fr, node] = W_lin @ x.T   (K = in-features)
            rhs_tiles = [rhs_loader(rhs_pool, ci, k, crows) for k in range(k_chunks)]
            tT = t_pool.tile([128, FR // 128, CHUNK], DT.bfloat16, tag="tT")
            for m in range(FR // 128):
                pt = ps1.tile([128, CHUNK], DT.float32, space="PSUM", tag="g1")
                for k in range(k_chunks):
                    nc.tensor.matmul(
                        out=pt[:, :crows],
                        lhsT=wT[:, k, m * 128 : (m + 1) * 128],
                        rhs=rhs_tiles[k][:, :crows],
                        start=(k == 0),
                        stop=(k == k_chunks - 1),
                    )
                nc.scalar.copy(out=tT[:, m, :crows], in_=pt[:, :crows])
            # G2 per 128-node block
            nblocks = (crows + 127) // 128
            for bi in range(nblocks):
                boff = bi * 128
                brows = min(128, crows - boff)
                hp = ps2.tile([128, HID], DT.float32, space="PSUM", tag="g2")
                nc.tensor.matmul(
                    out=hp[:brows, :],
                    lhsT=tT[:, 0, boff : boff + brows],
                    rhs=wsT[:, 0, :],
                    start=True,
                    stop=False,
                )
                nc.tensor.matmul(
                    out=hp[:brows, :],
                    lhsT=tT[:, 1, boff : boff + brows],
                    rhs=wsT[:, 1, :],
                    start=False,
                    stop=False,
                )
                nc.tensor.matmul(
                    out=hp[:brows, :],
                    lhsT=ones[0:1, 0:brows],
                    rhs=brow[0:1, :],
                    start=False,
                    stop=True,
                )
                fp = psf.tile([128, 2], DT.float32, space="PSUM", tag="gf")
                nc.tensor.matmul(
                    out=fp[:brows, :],
                    lhsT=tT[:, 0, boff : boff + brows],
                    rhs=apack[:, 0, :],
                    start=True,
                    stop=False,
                )
                nc.tensor.matmul(
                    out=fp[:brows, :],
                    lhsT=tT[:, 1, boff : boff + brows],
                    rhs=apack[:, 1, :],
                    start=False,
                    stop=False,
                )
                nc.tensor.matmul(
                    out=fp[:brows, :],
                    lhsT=ones[0:1, 0:brows],
                    rhs=fb[0:1, :],
                    start=False,
                    stop=True,
                )
                hb = asm_pool.tile([128, ROW], DT.bfloat16, tag="hb")
                nc.scalar.copy(out=hb[:brows, 0:HID], in_=hp[:brows, :])
                nc.vector.memset(hb[:brows, HID:ROW], 1.0)
                nc.vector.tensor_copy(
                    out=hb[:brows, HID + 1 : HID + 3], in_=fp[:brows, :]
                )
                f1c = asm_pool.tile([128, 1], DT.float32, tag="f1c")
                nc.vector.tensor_copy(out=f1c[:brows, :], in_=fp[:brows, 1:2])
                g0 = r0 + boff
                tdst = tableL if g0 < HALF else tableH
                goff = g0 if g0 < HALF else g0 - HALF
                nc.sync.dma_start(
                    out=tdst[goff : goff + brows, :], in_=hb[:brows, :]
                )
                nc.sync.dma_start(
                    out=f1full[g0 : g0 + brows, :], in_=f1c[:brows, :]
                )


def _edge_phase(tc, nc, cst, L, sched, x_in, tableL, tableH, f1tab, out_writer):
    """Per src-block: gather table rows by dst, build weighted one-hot S,
    accumulate [sum_w*h' | sum_w] in PSUM via matmul, normalize."""
    iota = cst["iota"]
    slotmat = cst["slotmat"]
    wrap_d = x_in["wrap"]
    srcwrap_d = x_in["srcwrap"]
    TMAX = max(s["t_lo"] + s["t_hi"] for s in sched)

    with (
        tc.tile_pool(name=f"ep{L}_g", bufs=3) as gpool,
        tc.tile_pool(name=f"ep{L}_i", bufs=3) as ipool,
        tc.tile_pool(name=f"ep{L}_c", bufs=3) as cpool,
        tc.tile_pool(name=f"ep{L}_s", bufs=2) as spool,
        tc.tile_pool(name=f"ep{L}_o", bufs=3) as opool,
        tc.tile_pool(name=f"ep{L}_ps", bufs=2, space="PSUM") as pspool,
    ):
        for b, sc in enumerate(sched):
            t_lo, t_hi, off = sc["t_lo"], sc["t_hi"], sc["off"]
            t_b = t_lo + t_hi
            if t_b == 0:
                continue
            brows = min(128, NLOC - b * 128)
            groups = []
            if t_lo:
                ilo = ipool.tile([128, 8 * t_lo], DT.int16, tag="ilo")
                nc.sync.dma_start(
                    out=ilo[:], in_=wrap_d[:, 8 * off : 8 * (off + t_lo)]
                )
                g_lo = gpool.tile([128, t_lo, ROW], DT.bfloat16, tag="glo")
                nc.gpsimd.dma_gather(
                    out_ap=g_lo[:],
                    in_ap=tableL.ap(),
                    idxs_ap=ilo[:],
                    num_idxs=t_lo * 128,
                    num_idxs_reg=t_lo * 128,
                    elem_size=ROW,
                    single_packet=False,
                    queue_num=(3 * b) % 4,
                )
                groups.append((g_lo, t_lo))
            if t_hi:
                ihi = ipool.tile([128, 8 * t_hi], DT.int16, tag="ihi")
                nc.sync.dma_start(
                    out=ihi[:], in_=wrap_d[:, 8 * (off + t_lo) : 8 * (off + t_b)]
                )
                g_hi = gpool.tile([128, t_hi, ROW], DT.bfloat16, tag="ghi")
                nc.gpsimd.dma_gather(
                    out_ap=g_hi[:],
                    in_ap=tableH.ap(),
                    idxs_ap=ihi[:],
                    num_idxs=t_hi * 128,
                    num_idxs_reg=t_hi * 128,
                    elem_size=ROW,
                    single_packet=False,
                    queue_num=(3 * b + 1) % 4,
                )
                groups.append((g_hi, t_hi))

            if1 = ipool.tile([128, 8 * t_b], DT.int16, tag="if1")
            nc.sync.dma_start(out=if1[:], in_=srcwrap_d[:, 8 * off : 8 * (off + t_b)])
            gf1 = gpool.tile([128, t_b, 128], DT.bfloat16, tag="gf1")
            nc.gpsimd.dma_gather(
                out_ap=gf1[:],
                in_ap=f1tab.ap(),
                idxs_ap=if1[:],
                num_idxs=t_b * 128,
                num_idxs_reg=t_b * 128,
                elem_size=128,
                single_packet=False,
                queue_num=(3 * b + 2) % 4,
            )
            f1sb = cpool.tile([128, TMAX], DT.float32, tag="f1sb")
            nc.scalar.copy(out=f1sb[:, 0:t_b], in_=gf1[:, :, 0])
            # coef = exp(lrelu(f1 + f2))
            logits = cpool.tile([128, TMAX], DT.float32, tag="logits")
            tcol = 0
            for g, tn in groups:
                nc.scalar.copy(
                    out=logits[:, tcol : tcol + tn], in_=g[:, :, HID + 1]
                )
                tcol += tn
            nc.vector.tensor_add(
                out=logits[:, 0:t_b], in0=logits[:, 0:t_b], in1=f1sb[:, 0:t_b]
            )
            coef = cpool.tile([128, TMAX], DT.float32, tag="coef")
            nc.vector.tensor_scalar(
                out=coef[:, 0:t_b], in0=logits[:, 0:t_b], scalar1=ALPHA,
                scalar2=None, op0=mybir.AluOpType.mult,
            )
            nc.vector.tensor_tensor(
                out=coef[:, 0:t_b], in0=coef[:, 0:t_b], in1=logits[:, 0:t_b],
                op=mybir.AluOpType.max,
            )
            nc.scalar.activation(
                out=coef[:, 0:t_b], in_=coef[:, 0:t_b],
                func=mybir.ActivationFunctionType.Exp,
            )

            # batched one-hot S for the whole block: S[:, t*128+j] =
            # (slot[p,t]==j) * coef[p,t]
            coefb = cpool.tile([128, TMAX], DT.bfloat16, tag="coefb")
            nc.vector.tensor_copy(out=coefb[:, 0:t_b], in_=coef[:, 0:t_b])
            S = spool.tile([128, t_b * 128], DT.bfloat16, tag="S")
            s_ap = S[:].rearrange("p (t j) -> p t j", t=t_b)
            sm = slotmat[:]
            slot_ap = bass.AP(
                sm.tensor, sm.offset + off, [list(sm.ap[0]), [1, t_b], [0, 128]]
            )
            io = iota[:]
            iota_ap = bass.AP(
                io.tensor, io.offset, [list(io.ap[0]), [0, t_b], [1, 128]]
            )
            nc.vector.tensor_tensor(
                out=s_ap, in0=slot_ap, in1=iota_ap, op=mybir.AluOpType.is_equal
            )
            cb = coefb[:]
            coef_ap = bass.AP(
                cb.tensor, cb.offset, [list(cb.ap[0]), [1, t_b], [0, 128]]
            )
            nc.vector.tensor_tensor(
                out=s_ap, in0=s_ap, in1=coef_ap, op=mybir.AluOpType.mult
            )

            acc = pspool.tile([128, HID + 1], DT.float32, space="PSUM", tag="acc")
            t_glob = 0
            for g, tn in groups:
                for t in range(tn):
                    nc.tensor.matmul(
                        out=acc[:],
                        lhsT=S[:, t_glob * 128 : (t_glob + 1) * 128],
                        rhs=g[:, t, 0 : HID + 1],
                        start=(t_glob == 0),
                        stop=(t_glob == t_b - 1),
                    )
                    t_glob += 1

            recip = opool.tile([128, 1], DT.float32, tag="recip")
            nc.vector.tensor_scalar(
                out=recip[:], in0=acc[:, HID : HID + 1], scalar1=1e-30,
                scalar2=None, op0=mybir.AluOpType.add,
            )
            nc.vector.reciprocal(out=recip[:], in_=recip[:])
            out_writer(opool, b, brows, acc, recip)


def _build_program(sched, TT):
    nc = bacc.Bacc(
        "TRN2", target_bir_lowering=False, debug=False, num_devices=CORES,
        enable_partition_id=True, num_swdge_queues=4,
    )
    x_in = {}

    def param(name, shape, dtype):
        x_in[name] = nc.declare_dram_parameter(name, list(shape), dtype, isOutput=False)

    param("xT", [IN, N], DT.bfloat16)
    param("wrap", [128, 8 * TT], DT.int16)
    param("srcwrap", [128, 8 * TT], DT.int16)
    param("slotmat", [128, TT], DT.bfloat16)
    param("w1t", [IN, FR], DT.bfloat16)
    param("w2t", [HID, FR], DT.bfloat16)
    for L in (1, 2):
        param(f"ws{L}t", [FR, HID], DT.bfloat16)
        param(f"apack{L}", [FR, 2], DT.bfloat16)
        param(f"brow{L}", [1, HID], DT.bfloat16)
        param(f"fb{L}", [1, 2], DT.bfloat16)
    param("wct", [HID, C], DT.bfloat16)
    param("bcrow", [1, C], DT.bfloat16)
    param("iota", [128, 128], DT.bfloat16)

    x2_out = nc.declare_dram_parameter("x2", [NLOC, HID], DT.float32, isOutput=True)
    cls_out = nc.declare_dram_parameter("cls", [NLOC, C], DT.float32, isOutput=True)

    # internal DRAM (own tensors: offset-0 for indirect/gather bases)
    tableL1 = nc.dram_tensor("tableL1", [HALF, ROW], DT.bfloat16)
    tableH1 = nc.dram_tensor("tableH1", [N - HALF, ROW], DT.bfloat16)
    tableL2 = nc.dram_tensor("tableL2", [HALF, ROW], DT.bfloat16)
    tableH2 = nc.dram_tensor("tableH2", [N - HALF, ROW], DT.bfloat16)
    f1full1 = nc.dram_tensor("f1full1", [N, 1], DT.float32)
    f1full2 = nc.dram_tensor("f1full2", [N, 1], DT.float32)
    F1ROWS = NBLK * 128
    f1tab1 = nc.dram_tensor("f1tab1", [F1ROWS, 128], DT.bfloat16)
    f1tab2 = nc.dram_tensor("f1tab2", [F1ROWS, 128], DT.bfloat16)
    x1_local = nc.dram_tensor("x1_local", [NLOC, HID], DT.bfloat16)
    x1_full = nc.dram_tensor("x1_full", [N, HID], DT.bfloat16)
    x2_bf = nc.dram_tensor("x2_bf", [NBLK * 128, HID], DT.bfloat16)

    with tile.TileContext(nc) as tc:
        pid = nc.partition_id()

        def localize_f1(L, f1full, f1tab):
            with tc.tile_pool(name=f"lc{L}", bufs=3) as lp:
                for b in range(NBLK):
                    brows = min(128, NLOC - b * 128)
                    f1c = lp.tile([128, 1], DT.float32, tag="f1c")
                    offv = nc.s_assert_within(
                        pid * NLOC + 128 * b,
                        128 * b,
                        (CORES - 1) * NLOC + 128 * b,
                        skip_runtime_assert=True,
                    )
                    nc.sync.dma_start(
                        out=f1c[:brows, :], in_=f1full[bass.ds(offv, brows), :]
                    )
                    rep = lp.tile([128, 128], DT.bfloat16, tag="rep")
                    if brows < 128:
                        nc.vector.memset(rep[:], 0.0)
                    nc.vector.tensor_copy(
                        out=rep[:brows, :],
                        in_=f1c[:brows, 0:1].to_broadcast([brows, 128]),
                    )
                    nc.sync.dma_start(
                        out=f1tab[b * 128 : (b + 1) * 128, :], in_=rep[:, :]
                    )

        with tc.tile_pool(name="const", bufs=1) as cstpool:
            cst = _load_consts(tc, nc, cstpool, x_in, TT)

            # ---- layer 1 table (replicated: all N rows) ----
            def rhs_l1(pool, ci, k, crows):
                t = pool.tile([128, CHUNK], DT.bfloat16, tag=f"rhs{k}")
                nc.sync.dma_start(
                    out=t[:, :crows],
                    in_=x_in["xT"][
                        k * 128 : (k + 1) * 128, ci * CHUNK : ci * CHUNK + crows
                    ],
                )
                return t

            _table_phase(
                tc, nc, cst, 1, rhs_l1, IN // 128, N, tableL1, tableH1, f1full1
            )
            localize_f1(1, f1full1, f1tab1)
            if PHASES < 2:
                return nc

            # ---- layer 1 edge phase -> x1 shard ----
            def write_x1(opool, b, brows, acc, recip):
                xb = opool.tile([128, HID], DT.bfloat16, tag="xb")
                nc.vector.tensor_scalar(
                    out=xb[:brows, :], in0=acc[:brows, 0:HID],
                    scalar1=recip[:brows, :], scalar2=None,
                    op0=mybir.AluOpType.mult,
                )
                nc.sync.dma_start(
                    out=x1_local[b * 128 : b * 128 + brows, :], in_=xb[:brows, :]
                )

            _edge_phase(tc, nc, cst, 1, sched, x_in, tableL1, tableH1, f1tab1, write_x1)
            if PHASES < 3:
                return nc

            # ---- all-gather x1 shards ----
            nc.gpsimd.collective_compute(
                "AllGather",
                mybir.AluOpType.bypass,
                replica_groups=[list(range(CORES))],
                ins=[x1_local.ap().opt()],
                outs=[x1_full.ap().opt()],
            )
            if PHASES < 4:
                return nc

            # ---- layer 2 table (replicated, from gathered x1) ----
            def rhs_l2(pool, ci, k, crows):
                t = pool.tile([128, CHUNK], DT.bfloat16, tag=f"rhs{k}")
                nc.sync.dma_start(
                    out=t[:, :crows],
                    in_=x1_full[
                        ci * CHUNK : ci * CHUNK + crows, k * 128 : (k + 1) * 128
                    ],
                    transpose=True,
                )
                return t

            _table_phase(
                tc, nc, cst, 2, rhs_l2, HID // 128, N, tableL2, tableH2, f1full2
            )
            localize_f1(2, f1full2, f1tab2)
            if PHASES < 5:
                return nc

            # ---- layer 2 edge phase -> x2 outputs ----
            def write_x2(opool, b, brows, acc, recip):
                xf = opool.tile([128, HID], DT.float32, tag="xf")
                nc.vector.tensor_scalar(
                    out=xf[:brows, :], in0=acc[:brows, 0:HID],
                    scalar1=recip[:brows, :], scalar2=None,
                    op0=mybir.AluOpType.mult,
                )
                nc.sync.dma_start(
                    out=x2_out[b * 128 : b * 128 + brows, :], in_=xf[:brows, :]
                )
                xb = opool.tile([128, HID], DT.bfloat16, tag="xb2")
                nc.vector.tensor_copy(out=xb[:brows, :], in_=xf[:brows, :])
                nc.sync.dma_start(
                    out=x2_bf[b * 128 : b * 128 + brows, :], in_=xb[:brows, :]
                )

            _edge_phase(tc, nc, cst, 2, sched, x_in, tableL2, tableH2, f1tab2, write_x2)
            if PHASES < 6:
                return nc

            # ---- classifier ----
            with (
                tc.tile_pool(name="cl_x", bufs=3) as clx,
                tc.tile_pool(name="cl_o", bufs=3) as clo,
                tc.tile_pool(name="cl_ps", bufs=2, space="PSUM") as clps,
            ):
                if NLOC % 128:
                    zt = clx.tile([128, HID], DT.bfloat16, tag="zt")
                    nc.vector.memset(zt[:], 0.0)
                    nc.sync.dma_start(
                        out=x2_bf[NLOC : NBLK * 128, :],
                        in_=zt[0 : NBLK * 128 - NLOC, :],
                    )
                wct = cst["wct"]
                bcrow = cst["bcrow"]
                ones = cst["ones"]
                for b in range(NBLK):
                    brows = min(128, NLOC - b * 128)
                    brows16 = (brows + 15) // 16 * 16
                    cp = clps.tile([128, C], DT.float32, space="PSUM", tag="cp")
                    for k in range(HID // 128):
                        xt = clx.tile([128, 128], DT.bfloat16, tag="xt")
                        nc.sync.dma_start(
                            out=xt[:, 0:brows16],
                            in_=x2_bf[
                                b * 128 : b * 128 + brows16, k * 128 : (k + 1) * 128
                            ],
                            transpose=True,
                        )
                        nc.tensor.matmul(
                            out=cp[:brows, :],
                            lhsT=xt[:, 0:brows],
                            rhs=wct[:, k, :],
                            start=(k == 0),
                            stop=False,
                        )
                    nc.tensor.matmul(
                        out=cp[:brows, :],
                        lhsT=ones[0:1, 0:brows],
                        rhs=bcrow[0:1, :],
                        start=False,
                        stop=True,
                    )
                    co = clo.tile([128, C], DT.float32, tag="co")
                    nc.vector.tensor_copy(out=co[:brows, :], in_=cp[:brows, :])
                    nc.sync.dma_start(
                        out=cls_out[b * 128 : b * 128 + brows, :], in_=co[:brows, :]
                    )

    return nc


# ---------------------------------------------------------------------------
# Entry point
# ---------------------------------------------------------------------------

_RUN_KWARGS = {}  # test.py can inject trace=True


def kernel(**inputs):
    x = np.asarray(inputs["x"], np.float32)
    edges = np.asarray(inputs["edges"])

    sched, TT, metas = _prep_edges(edges)
    wts = _prep_weights(inputs)
    xT = np.ascontiguousarray(x.T).astype(BF16)

    nc = _build_program(sched, TT)
    nc.compile()

    shared = {k: v for k, v in wts.items()}
    shared["xT"] = xT
    in_maps = []
    for c in range(CORES):
        m = dict(shared)
        m.update(metas[c])
        in_maps.append(m)

    res = run_bass_kernel_spmd(
        nc, in_maps, core_ids=list(range(CORES)), **_RUN_KWARGS
    )
    kernel._last_results = res

    x2 = np.concatenate(
        [np.asarray(res.results[c]["x2"], np.float32) for c in range(CORES)], axis=0
    )
    out = np.concatenate(
        [np.asarray(res.results[c]["cls"], np.float32) for c in range(CORES)], axis=0
    )
    return (out, x2)
